# revision 24
# baseline (speedup 1.0000x reference)
"""Sparse avg-pool (segment mean) for Trainium2, 8 NeuronCores.

Strategy: range-shard the coarse-voxel id space across the 8 cores (core k owns
coarse ids [k*31360, (k+1)*31360)).  The host routes each fine voxel row to the
core owning its coarse id (the sharding step), so no inter-core collective is
needed: every core owns a disjoint slice of the output.

On each core the segment-sum runs entirely on-chip: fine rows (64 feats + a
ones column for the count) are streamed into SBUF and accumulated with the
GPSIMD `dma_scatter_add` custom DMA instruction in SBUF-destination mode
(parity-split CCE add), so the [31360, 66] accumulator lives in SBUF and the
only HBM traffic is the one-time streaming read of the shard.  A final pass
computes sums/max(count,1) and DMAs the per-core output slice out.

Hardware constraint (verified on silicon): duplicate indices within one
dma_scatter_add call race in the CCE read-modify-write and corrupt/crash.
The host therefore sorts each core's tokens by local id and deals them
round-robin over the calls (call c takes sorted tokens [c::n_call]); an id's
occurrences are consecutive after the sort, so they land in distinct calls
(max per-id count must be <= n_call, asserted; it is ~Poisson(8) here).

Call padding (PAD_MODE):
  "trash": pads scatter to distinct throwaway accumulator slots (ids
           31744+j, groups 124..127); num_idxs_reg is the static t_call.
  "neg":   pads are idx=-1 tails skipped by the ucode; num_idxs_reg is
           still the static t_call (the ucode ignores the -1 tail).

Accumulator layout (sbuf_tokens_per_rank=128, parity_reg=0): local id v with
row p = v & 127, slot s = v >> 7 lands in acc_even (s even) or acc_odd (s odd)
at [p, s >> 1, :].
"""
import os
import sys
from dataclasses import dataclass

sys.path.insert(0, "/opt/trn_rl_repo")

import numpy as np

NCORES = 8
C = 64
# "neg": idx=-1 tail pads, skipped by the ucode (HW-verified); CoreSim's
#        stricter num_idxs_reg assert rejects it, so sim tests use "trash".
# "trash": pads scatter to distinct throwaway accumulator slots.
PAD_MODE = os.environ.get("KERNEL_PAD_MODE", "trash")
# "f32" or "bf16": dtype of the streamed tokens and the SBUF accumulator.
# bf16 halves the SDMA fabric traffic; counts stay exact (integers <= 256 in
# bf16) and the divide runs in f32 after an upconvert.
ACC_DTYPE = os.environ.get("KERNEL_ACC_DTYPE", "f32")


@dataclass(frozen=True)
class Cfg:
    n_coarse_pad: int = 250_880   # 8 * 245 * 128
    s_tot: int = 253_952          # padded tokens per core (128*1984, 16*15872)
    # tokens per dma_scatter_add call; 8192 overflows the 16KB SWDGE
    # descriptor carveout and crashes the device (HW-bisected: 4096 ok).
    # 3968 keeps per-call pad count <= 166 (fits the 256 trash slots).
    t_call: int = 3968
    cw: int = 66                  # 64 feats + count + pad

    @property
    def rng(self):  # coarse ids per core
        return self.n_coarse_pad // NCORES

    @property
    def n_slot(self):  # 128-id output slots per core
        return self.rng // 128

    @property
    def ngroup(self):  # even-parity groups (= ceil(n_slot/2))
        return (self.n_slot + 1) // 2

    @property
    def ngroup_odd(self):
        return self.n_slot // 2

    @property
    def ng_acc(self):  # accumulator group columns: data + 1 trash group (123)
        return self.ngroup + 1

    @property
    def trash_base(self):  # first trash id: slot 2*ngroup (spare group, even)
        return 128 * (2 * self.ngroup)

    @property
    def n_trash(self):  # distinct trash ids (slots 246+247: group 123 e/o)
        return 2 * 128

    @property
    def s_slots(self):
        return self.s_tot // 128

    @property
    def n_call(self):
        return self.s_tot // self.t_call

    @property
    def slots_call(self):
        return self.t_call // 128


CFG = Cfg()
_nc_cache = {}
LAST_RESULT = None


def build_nc(cfg: Cfg):
    from concourse import bacc, mybir, tile

    fdt = mybir.dt.bfloat16 if ACC_DTYPE == "bf16" else mybir.dt.float32
    nc = bacc.Bacc("TRN2", target_bir_lowering=False)
    feats_ext = nc.declare_dram_parameter(
        "feats", [128, cfg.s_slots, cfg.cw], fdt, isOutput=False
    )
    idxs_ext = nc.declare_dram_parameter(
        "idxs", [128, cfg.s_tot // 16], mybir.dt.int16, isOutput=False
    )
    # out_e[p, g, :] = mean row of local id (2g)*128 + p (even slots);
    # out_o[p, g, :] = local id (2g+1)*128 + p.  Host de-interleaves.
    out_e_ext = nc.declare_dram_parameter(
        "out_e", [128, cfg.ngroup, C], mybir.dt.float32, isOutput=True
    )
    out_o_ext = nc.declare_dram_parameter(
        "out_o", [128, cfg.ngroup_odd, C], mybir.dt.float32, isOutput=True
    )

    with tile.TileContext(nc) as tc:
        with (
            tc.tile_pool(name="acc", bufs=1) as accp,
            tc.tile_pool(name="stage", bufs=2) as stagep,
            tc.tile_pool(name="idx", bufs=2) as idxp,
            tc.tile_pool(name="fin", bufs=1) as finp,
        ):
            acc_e = accp.tile([128, cfg.ng_acc, cfg.cw], fdt)
            acc_o = accp.tile([128, cfg.ng_acc, cfg.cw], fdt)
            nc.vector.memset(acc_e[:], 0.0)
            nc.vector.memset(acc_o[:], 0.0)

            max_calls = int(os.environ.get("KERNEL_MAX_CALLS", str(cfg.n_call)))
            for c in range(min(cfg.n_call, max_calls)):
                src = stagep.tile([128, cfg.slots_call, cfg.cw], fdt)
                idx_t = idxp.tile([128, cfg.t_call // 16], mybir.dt.int16)
                nc.sync.dma_start(
                    out=src[:],
                    in_=feats_ext[:, c * cfg.slots_call : (c + 1) * cfg.slots_call, :],
                )
                nc.sync.dma_start(
                    out=idx_t[:],
                    in_=idxs_ext[
                        :, c * (cfg.t_call // 16) : (c + 1) * (cfg.t_call // 16)
                    ],
                )
                nc.gpsimd.dma_scatter_add(
                    acc_e[:],
                    src[:],
                    idx_t[:],
                    cfg.t_call,
                    cfg.t_call,
                    cfg.cw,
                    sbuf_tokens_per_rank=128,
                    parity_reg=0,
                    out_ap_other=acc_o[:],
                )

            # divide: out = sums / max(cnt, 1); empty slots give 0/1 = 0.
            # Per-group loop: tensor_scalar ops broadcast a [128,1] scalar
            # along the free dim (known-good DVE pattern).
            for par, acc, ng, out_ext in (
                (0, acc_e, cfg.ngroup, out_e_ext),
                (1, acc_o, cfg.ngroup_odd, out_o_ext),
            ):
                if ACC_DTYPE == "bf16":
                    accf = finp.tile(
                        [128, cfg.ngroup, cfg.cw], mybir.dt.float32, tag="accf"
                    )
                    nc.vector.tensor_copy(accf[:, :ng, :], acc[:, :ng, :])
                    acc = accf
                den = finp.tile([128, ng, 1], mybir.dt.float32, tag=f"den{par}")
                nc.vector.tensor_scalar_max(den[:], acc[:, :ng, C : C + 1], 1.0)
                inv = finp.tile([128, ng, 1], mybir.dt.float32, tag=f"inv{par}")
                nc.vector.reciprocal(inv[:], den[:])
                ot = finp.tile([128, ng, C], mybir.dt.float32, tag=f"ot{par}")
                nc.vector.tensor_tensor(
                    out=ot[:],
                    in0=acc[:, :ng, :C],
                    in1=inv[:].to_broadcast([128, ng, C]),
                    op=mybir.AluOpType.mult,
                )
                nc.sync.dma_start(out=out_ext[:], in_=ot[:])
    nc.compile()
    return nc


def shard_inputs(feats, ids, cfg: Cfg):
    """Host sharding: route rows to owner cores, build device layouts."""
    ids = np.asarray(ids, dtype=np.int64).ravel()
    feats = np.asarray(feats, dtype=np.float32)
    owner = ids // cfg.rng
    local = (ids - owner * cfg.rng).astype(np.int16)
    order = np.argsort(owner, kind="stable")
    counts = np.bincount(owner, minlength=NCORES)
    assert counts.max() <= cfg.s_tot, f"shard overflow: {counts.max()} > {cfg.s_tot}"
    offs = np.zeros(NCORES + 1, np.int64)
    np.cumsum(counts, out=offs[1:])
    feats_sorted = feats[order]
    local_sorted = local[order]

    if ACC_DTYPE == "bf16":
        import ml_dtypes

        np_fdt = ml_dtypes.bfloat16
    else:
        np_fdt = np.float32

    in_maps = []
    for k in range(NCORES):
        n_k = int(counts[k])
        fk = feats_sorted[offs[k] : offs[k + 1]]
        lk = local_sorted[offs[k] : offs[k + 1]]
        fa = np.zeros((cfg.s_tot, cfg.cw), np_fdt)
        idv = np.full(cfg.s_tot, -1, np.int16)
        if PAD_MODE == "trash":
            # pads: throwaway ids; tail pads of each call are distinct because
            # consecutive positions mod n_trash are distinct for <= n_trash pads
            pad_pat = (cfg.trash_base + np.arange(cfg.n_trash)).astype(np.int16)
            idv[:] = np.resize(pad_pat, cfg.s_tot)
        if n_k:
            # sort by id; deal round-robin over calls -> per-call distinct ids
            sorder = np.argsort(lk, kind="stable")
            per_id = np.bincount(lk, minlength=1)
            assert per_id.max() <= cfg.n_call, (
                f"id multiplicity {per_id.max()} exceeds n_call={cfg.n_call}"
            )
            for cidx in range(cfg.n_call):
                sel = sorder[cidx :: cfg.n_call]
                sz = sel.shape[0]
                base = cidx * cfg.t_call
                assert sz <= cfg.t_call
                if PAD_MODE == "trash":
                    assert cfg.t_call - sz <= cfg.n_trash, (
                        f"too many pads in call {cidx}: {cfg.t_call - sz}"
                    )
                fa[base : base + sz, :C] = fk[sel]
                fa[base : base + sz, C] = 1.0
                idv[base : base + sz] = lk[sel]
        arranged = np.ascontiguousarray(
            fa.reshape(cfg.s_slots, 128, cfg.cw).transpose(1, 0, 2)
        )
        wrapped = np.ascontiguousarray(idv.reshape(cfg.s_tot // 16, 16).T)
        idx_full = np.ascontiguousarray(np.tile(wrapped, (8, 1)))
        in_maps.append({"feats": arranged, "idxs": idx_full})
    return in_maps


def assemble_output(results, n_coarse, cfg: Cfg):
    out = np.empty((NCORES * cfg.rng, C), np.float32)
    for k in range(NCORES):
        oe = results[k]["out_e"].reshape(128, cfg.ngroup, C)
        oo = results[k]["out_o"].reshape(128, cfg.ngroup_odd, C)
        blk = out[k * cfg.rng : (k + 1) * cfg.rng].reshape(cfg.n_slot, 128, C)
        blk[0::2] = oe.transpose(1, 0, 2)
        blk[1::2] = oo.transpose(1, 0, 2)
    return out[:n_coarse]


def emulate_device(in_map, cfg: Cfg):
    """Numpy emulation of the device kernel for one core (for testing)."""
    feats = in_map["feats"]  # [128, s_slots, cw]
    idxs = in_map["idxs"]  # [128, s_tot//16]
    acc_e = np.zeros((128, cfg.ng_acc, cfg.cw), np.float64)
    acc_o = np.zeros((128, cfg.ng_acc, cfg.cw), np.float64)
    unwrapped = idxs[:16, :].T.ravel().astype(np.int64)  # token i at [i%16, i//16]
    for cidx in range(cfg.n_call):
        base = cidx * cfg.t_call
        seen = set()
        for j in range(cfg.t_call):
            i = base + j
            v = int(unwrapped[i])
            if v < 0:
                continue
            assert v not in seen, f"dup id {v} within call {cidx}"
            seen.add(v)
            row, slot = v & 127, v >> 7
            g, par = slot >> 1, slot & 1
            assert g < cfg.ng_acc, (v, g)
            (acc_e if par == 0 else acc_o)[row, g, :] += feats[i % 128, i // 128, :]
    res = {}
    for name, acc, ng in (
        ("out_e", acc_e, cfg.ngroup),
        ("out_o", acc_o, cfg.ngroup_odd),
    ):
        den = np.maximum(acc[:, :ng, C], 1.0)[:, :, None]
        res[name] = (acc[:, :ng, :C] / den).astype(np.float32)
    return res


def _install_axon_hooks_shim():
    """Provide antenv.axon_hooks + the ctypes NTFF hook if the image lacks it.

    Mirrors trn_agent_boot.trn_boot._ntff_profile_via_ctypes so that
    run_bass_kernel_spmd(trace=True) can profile under axon.
    """
    import contextlib
    import ctypes
    import types

    try:
        from antenv.axon_hooks import get_axon_ntff_profile_hook  # noqa: F401

        return
    except ImportError:
        pass
    import antenv

    mod = types.ModuleType("antenv.axon_hooks")
    state = {"h": None}
    mod.set_axon_ntff_profile_hook = lambda h: state.__setitem__("h", h)
    mod.get_axon_ntff_profile_hook = lambda: state["h"]
    antenv.axon_hooks = mod
    sys.modules["antenv.axon_hooks"] = mod

    so_path = "/opt/axon/libaxon_pjrt.so"
    if not os.path.exists(so_path):
        return
    lib = ctypes.CDLL(so_path)
    if not hasattr(lib, "axon_start_nrt_profile"):
        return
    lib.axon_start_nrt_profile.argtypes = [
        ctypes.POINTER(ctypes.c_int64),
        ctypes.c_size_t,
    ]
    lib.axon_start_nrt_profile.restype = ctypes.c_int64
    lib.axon_stop_nrt_profile.argtypes = [ctypes.c_char_p]
    lib.axon_stop_nrt_profile.restype = ctypes.c_int64

    @contextlib.contextmanager
    def _hook(output_dir, device_ids):
        import jax

        jax.devices()
        if device_ids:
            ids = (ctypes.c_int64 * len(device_ids))(*device_ids)
            rc = lib.axon_start_nrt_profile(ids, len(device_ids))
        else:
            rc = lib.axon_start_nrt_profile(None, 0)
        if rc != 0:
            raise RuntimeError(f"axon_start_nrt_profile rc={rc}")
        try:
            yield
        finally:
            n = lib.axon_stop_nrt_profile(str(output_dir).encode())
            print(f"profile: {n} file(s) written to {output_dir}", file=sys.stderr)

    state["h"] = _hook


def kernel(fine_feats, coarse_ids, num_coarse):
    global LAST_RESULT
    from concourse.bass_utils import run_bass_kernel_spmd

    cfg = CFG
    in_maps = shard_inputs(fine_feats, coarse_ids, cfg)

    if "full" not in _nc_cache:
        _nc_cache["full"] = build_nc(cfg)
    nc = _nc_cache["full"]

    trace = bool(int(os.environ.get("KERNEL_TRACE", "0")))
    if trace:
        _install_axon_hooks_shim()
    res = run_bass_kernel_spmd(nc, in_maps, core_ids=list(range(NCORES)), trace=trace)
    LAST_RESULT = res
    return assemble_output(res.results, int(num_coarse), cfg)
